# revision 18
# baseline (speedup 1.0000x reference)
"""Causal self-attention (B=2,T=2048,C=1024,H=16) on 8 trn2 NeuronCores.

Sharding: core c handles batch b=c//4 and 4 heads (c%4)*4..+4 (tensor-parallel
over heads x data-parallel over batch).

v2: fp8 DoubleRow matmuls where numerics allow (residual-split operands):
  stage A (qk and v): 3-product fp8-DR  x_hi@W_hi + x_hi@W_lo + x_lo@W_hi
    (x split host-side into fp8 hi+lo; W pre-scaled to unit rms and split).
    Per-group pow2 scales undone in the drain (tensor_scalar mult+add).
  scores: bf16 as before (fp8 there breaks the 2e-2 tolerance), with per-ki
    off-trim; causal mask applied by accumulating a -80 triangular tile into
    PSUM via a tiny identity-stationary matmul (PE) instead of es*mask on
    DVE/Pool.
  exp: Act engine, bias -3 (fp8 range headroom), es written as fp8e4m3.
  AV: DoubleRow pairs (es,es) stride-0 stationary x (v_hi,v_lo) moving;
    V drained as fp8 hi+lo split (exact to ~0.1%); l ones-column is a
    constant region of the V tiles (bv folded into b_proj host-side).
  proj: bf16 unchanged.  y normalize/transpose/tail: unchanged.
"""
import sys

sys.path.insert(0, "/opt/trn_rl_repo")

import numpy as np
import ml_dtypes

import concourse.bass as bass
import concourse.mybir as mybir
import concourse.tile as tile
from concourse import bacc
from concourse.bass_utils import run_bass_kernel_spmd

B, T, C, H, HD = 2, 2048, 1024, 16, 64
NCORES = 8
HPC = 4            # heads per core
CT = C // 128      # 8 contraction tiles
CTP = CT // 2      # 4 contraction-tile pairs (DoubleRow)
TJ = T // 512      # 4 q chunks
TT = T // 128      # 16 tok tiles
VW = 256           # V matmul cols per core (4 heads x 64, l-cols separate)
F32 = mybir.dt.float32
BF = mybir.dt.bfloat16
FP8 = mybir.dt.float8e4
EXP = mybir.ActivationFunctionType.Exp
DR = mybir.MatmulPerfMode.DoubleRow
NFP8 = ml_dtypes.float8_e4m3

SQ = 256.0         # wq fp8 pre-scale (alpha*Wq ~ 1/256 rms)
SK = 32.0          # wk fp8 pre-scale
SV = 32.0          # wv fp8 pre-scale
EB = 3.0           # exp bias (es = exp(s-3), max ~137 < fp8 240)

_CACHE = {}


def _emit(tc, nc, d):
    (d_xhi, d_xlo, d_wqh, d_wql, d_wvh, d_wvl, d_wp, d_bqk, d_tri, d_ident,
     d_out) = d
    from contextlib import ExitStack
    with tc.tile_pool(name="const", bufs=1) as pc, \
         tc.tile_pool(name="qk", bufs=1) as pqk, \
         tc.tile_pool(name="vv", bufs=1) as pvv, \
         tc.tile_pool(name="yt", bufs=1) as pyt, \
         tc.tile_pool(name="w_in", bufs=1) as pw, \
         tc.tile_pool(name="x_in", bufs=1) as px, \
         tc.tile_pool(name="fill", bufs=2, space="PSUM") as pfill, \
         tc.tile_pool(name="ex", bufs=4) as pex, \
         tc.tile_pool(name="nrm", bufs=4) as pn, \
         tc.tile_pool(name="ysb", bufs=8) as pysb, \
         tc.tile_pool(name="po", bufs=4) as po:
        inner = ExitStack()
        psS = inner.enter_context(tc.tile_pool(name="psS", bufs=2, space="PSUM"))
        psY = inner.enter_context(tc.tile_pool(name="psY", bufs=2, space="PSUM"))
        bqk = pc.tile([128, 4], F32, tag="bqk")
        tri = pc.tile([128, 128], FP8, tag="tri")
        ident = pc.tile([128, 128], FP8, tag="ident")
        negeb = pc.tile([128, 1], F32, tag="negeb")
        warm = pc.tile([128, 512], BF, tag="warm")
        nc.gpsimd.memset(negeb[:], -EB)
        nc.gpsimd.memset(warm[:], 0.0)
        # preload the Exp activation table during the input-DMA wait so the
        # first real exp doesn't pay the 1.3us table load
        wes = pex.tile([128, 2, 512], FP8, tag="es", name="warmes")
        nc.scalar.activation(wes[0:1, 0, 0:1], negeb[0:1, :], EXP, bias=0.0)

        qkT = [pqk.tile([128, T], BF, tag=f"qk{i}", name=f"qkT{i}") for i in range(4)]
        # V tiles: [128 tok, 2(hi/lo), 4 heads, 65]; col 64 of each head block
        # is the constant l-column (hi=1, lo=0), set once below.
        V = [pvv.tile([128, 2, HPC, 65], FP8, tag=f"v{i}", name=f"V{i}")
             for i in range(TT)]
        yT = [pyt.tile([128, T], BF, tag=f"y{i}", name=f"yT{i}") for i in range(2)]
        wqh = pw.tile([128, 4, CTP, 2, 128], FP8, tag="wqh")
        wql = pw.tile([128, 4, CTP, 2, 128], FP8, tag="wql")
        wvh = pw.tile([128, CTP, 2, VW], FP8, tag="wvh")
        wvl = pw.tile([128, CTP, 2, VW], FP8, tag="wvl")
        wp = pc.tile([128, 2 * C], BF, tag="wp")
        xhi = px.tile([128, TJ, CTP, 2, 512], FP8, tag="xhi")
        xlo = px.tile([128, TJ, CTP, 2, 512], FP8, tag="xlo")

        # input DMAs, ordered along the startup critical chain: consts, then
        # the operands of a_qk(0,0/1) in product order, then a_v's weights,
        # then the rest
        nc.sync.dma_start(bqk[:], d_bqk)
        nc.sync.dma_start(tri[:], d_tri)
        nc.sync.dma_start(ident[:], d_ident)
        nc.sync.dma_start(wqh[:, 0], d_wqh[:, 0])
        nc.sync.dma_start(xhi[:, 0], d_xhi[:, 0])
        nc.sync.dma_start(wql[:, 0], d_wql[:, 0])
        nc.sync.dma_start(wqh[:, 1], d_wqh[:, 1])
        nc.sync.dma_start(wql[:, 1], d_wql[:, 1])
        nc.sync.dma_start(xlo[:, 0], d_xlo[:, 0])
        nc.sync.dma_start(wvh[:], d_wvh)
        nc.sync.dma_start(wvl[:], d_wvl)
        for mo in range(2, 4):
            nc.sync.dma_start(wqh[:, mo], d_wqh[:, mo])
            nc.sync.dma_start(wql[:, mo], d_wql[:, mo])
        for tj in range(1, TJ):
            nc.sync.dma_start(xhi[:, tj], d_xhi[:, tj])
            nc.sync.dma_start(xlo[:, tj], d_xlo[:, tj])
        nc.sync.dma_start(wp[:], d_wp)

        # constant l-columns of the V tiles (hi=1 -> l = sum es; lo=0)
        for tt in range(TT):
            nc.gpsimd.memset(V[tt][:, 0, :, 64:65], 1.0)
            nc.gpsimd.memset(V[tt][:, 1, :, 64:65], 0.0)

        # PE p-state warmup during the input-DMA wait (results unused)
        ws = psS.tile([128, 2, 512], F32, tag="s", name="warms")
        for i in range(8):
            nc.tensor.matmul(ws[0:64, 0, :], warm[:, :64], warm[:, :],
                             start=True, stop=True)

        # ---------------- stage A blocks (emitted via filler queue) -------
        # 3-product fp8 DoubleRow: xh@Wh + xh@Wl + xl@Wh (x/W host-split).
        def a_qk(tj, mo):
            sc_ = (1.0 / SQ) if mo % 2 == 0 else (1.0 / SK)

            def emit():
                ps = pfill.tile([128, 512], F32, tag="fill", name=f"psqk{tj}_{mo}")
                n = 0
                for wt, xt in ((wqh, xhi), (wql, xhi), (wqh, xlo)):
                    for cp in range(CTP):
                        nc.tensor.matmul(
                            ps[:], wt[:, mo, cp], xt[:, tj, cp],
                            start=(n == 0), stop=(n == 11), perf_mode=DR)
                        n += 1
                nc.vector.tensor_scalar(
                    qkT[mo][:, tj * 512:(tj + 1) * 512], ps[:],
                    sc_, bqk[:, mo:mo + 1],
                    mybir.AluOpType.mult, mybir.AluOpType.add)
            return emit

        def a_v(tt):
            tj, ti = divmod(tt, 4)

            def emit():
                psv = pfill.tile([128, 512], F32, tag="fill", name=f"psv{tt}")
                n = 0
                for wt, xt in ((wvh, xhi), (wvl, xhi), (wvh, xlo)):
                    for cp in range(CTP):
                        nc.tensor.matmul(
                            psv[:, :VW],
                            xt[:, tj, cp, :, ti * 128:(ti + 1) * 128],
                            wt[:, cp],
                            start=(n == 0), stop=(n == 11), perf_mode=DR)
                        n += 1
                # split drain: hi = fp8(psv/SV), lo = fp8(psv/SV - hi)
                nc.vector.tensor_scalar(
                    V[tt][:, 0, :, 0:64], psv[:, :VW], 1.0 / SV, None,
                    mybir.AluOpType.mult)
                nc.vector.scalar_tensor_tensor(
                    V[tt][:, 1, :, 0:64], psv[:, :VW], 1.0 / SV,
                    V[tt][:, 0, :, 0:64],
                    mybir.AluOpType.mult, mybir.AluOpType.subtract)
            return emit

        def proj(qj, mo, pool=None, ptag="fill", act_copy=False):
            def emit():
                pps = (pool or pfill).tile([128, 512], F32, tag=ptag,
                                           name=f"pps{qj}_{mo}")
                for kt2 in range(2):
                    nc.tensor.matmul(
                        pps[:],
                        wp[:, kt2 * C + mo * 128:kt2 * C + (mo + 1) * 128],
                        yT[kt2][:, qj * 512:(qj + 1) * 512],
                        start=(kt2 == 0), stop=(kt2 == 1))
                ot = po.tile([128, 512], BF, tag="ot")
                if act_copy:
                    nc.scalar.activation(
                        ot[:], pps[:], mybir.ActivationFunctionType.Copy)
                else:
                    nc.vector.tensor_copy(ot[:], pps[:])
                nc.sync.dma_start(
                    d_out[:, mo, qj * 512:(qj + 1) * 512], ot[:])
            return emit

        # all stage A goes through the deadline queue (tj=0 included): the
        # first-needed blocks are forced before unit 0's scores

        # -------- attention: one global pair pipeline across sections -----
        units = []
        for qj, hp in [(0, 0), (0, 1), (1, 0), (1, 1),
                       (2, 0), (3, 0), (2, 1), (3, 1)]:
            for lh in range(2):
                for p in range(2 * qj + 2):
                    units.append((qj, hp, lh, p))

        # ---- filler scheduling: deadline-driven ----
        # Each stage-A block is FORCED just before the first unit whose
        # score/AV emission needs it (one unit early, hiding the DVE drain
        # latency); otherwise blocks are emitted earliest-deadline-first
        # while the emitted-PE-work clock trails the emitted-Act-work clock,
        # so the PE never idles while late Act-bound windows still get their
        # just-in-time share of stage-A work.
        def unit_deadlines():
            """deadline[block] = first unit index needing it."""
            dl_qk = {}
            dl_v = {}
            for i, (qj, hp, lh, p) in enumerate(units):
                for mo in (2 * hp, 2 * hp + 1):
                    dl_qk.setdefault((qj, mo), i)
                for ki in range(2):
                    kt = 2 * p + ki
                    if kt <= 4 * qj + 3:
                        dl_v.setdefault((qj, kt), i)
            return dl_qk, dl_v

        dl_qk, dl_v = unit_deadlines()
        fq = []
        for tj in range(TJ):
            for mo in range(4):
                fq.append([dl_qk[(tj, mo)], a_qk(tj, mo), 1280])
            for tt in range(4 * tj, 4 * tj + 4):
                fq.append([dl_v[(tj, tt)], a_v(tt), 640])
        fq.sort(key=lambda e: e[0])

        clock = {"pe": 0.0}

        def force_filler(i):
            """emit every block whose first-use unit is <= unit i"""
            while fq and fq[0][0] <= i:
                e = fq.pop(0)
                e[1]()
                clock["pe"] += e[2]

        def drain_filler(limit):
            """emit filler until the emitted-PE clock reaches `limit`"""
            while fq and clock["pe"] < limit:
                e = fq.pop(0)
                e[1]()
                clock["pe"] += e[2]

        ysb_tiles = {}
        state = {}

        def sc(u):
            """Scores for unit u: per-ki trimmed matmul + PE causal mask."""
            qj, hp, lh, p = u
            s = psS.tile([128, 2, 512], F32, tag="s")
            es = pex.tile([128, 2, 512], FP8, tag="es")
            for ki in range(2):
                kt = 2 * p + ki
                r = kt - 4 * qj
                off = 128 * r if r >= 0 else 0
                nc.tensor.matmul(
                    s[:, ki, off:512],
                    qkT[2 * hp + 1][64 * lh:64 * lh + 64, kt * 128:(kt + 1) * 128],
                    qkT[2 * hp][64 * lh:64 * lh + 64,
                                qj * 512 + off:(qj + 1) * 512],
                    start=True, stop=(r < 0))
                if r >= 0:
                    # causal mask: accumulate -80 lower-triangle into the
                    # diagonal tile (exp then yields ~0; no DVE/Pool mask mul)
                    nc.tensor.matmul(
                        s[:, ki, off:off + 128], ident[:], tri[:],
                        start=False, stop=True, skip_group_check=True)
                clock["pe"] += (512 - off) * 0.4167 + (53.3 if r >= 0 else 0)
            return s, es

        pend = {}
        for i, u in enumerate(units):
            qj, hp, lh, p = u
            npair = 2 * qj + 2
            h_loc = 2 * hp + lh
            if i == 0:
                force_filler(3)
                pend[0] = sc(u)
            wstart = clock["pe"]
            if i + 1 < len(units):
                force_filler(i + 3)
                pend[i + 1] = sc(units[i + 1])
            s, es = pend.pop(i)
            off = 256 if p == 2 * qj + 1 else 0
            nc.scalar.activation(
                es[:, :, off:512], s[:, :, off:512], EXP, bias=negeb[:])
            # per-window pacing: fill this unit's Act span (minus the AV
            # work still to come) with stage-A/proj filler, no cross-window
            # debt so late Act-bound windows still get their share
            act_c = (1024 - 2 * off) * 0.833 + 185
            av_c = 13.5 * sum(1 for qt in range(4) for ki in range(2)
                              if 2 * p + ki <= 4 * qj + qt)
            drain_filler(wstart + act_c - av_c - 200)
            # AV (transposed, DoubleRow): psy[qt] += es_kt(qt-slice) @ (vhi|vlo)
            key = (qj, hp, lh)
            if key not in state:
                state[key] = [psY.tile([128, 512], F32, tag="psy",
                                       name=f"psy{qj}_{hp}_{lh}"), True]
            psy, first_mm = state[key]
            for qt in range(4):
                for ki in range(2):
                    kt = 2 * p + ki
                    if kt > 4 * qj + qt:
                        continue
                    es_pair = es[:, ki, qt * 128:(qt + 1) * 128] \
                        .unsqueeze(1).broadcast_to([128, 2, 128])
                    nc.tensor.matmul(
                        psy[:, qt * 128:qt * 128 + 65],
                        es_pair,
                        V[kt][:, :, h_loc, :],
                        start=first_mm,
                        stop=(kt == 4 * qj + qt),
                        perf_mode=DR,
                        skip_group_check=True)
                    first_mm = False
                    clock["pe"] += 13.5
            state[key][1] = first_mm
            if (qj, hp, lh) == (3, 1, 1) and p >= 6:
                # epilogue fast-path: normalize + transpose each qt as soon
                # as its AV accumulation stops so only qt2/qt3 trail the
                # final exp
                qts = (0, 1) if p == 6 else (2, 3)
                y_sb = ysb_tiles[qj]
                rc = pn.tile([128, 4], F32, tag="rc", name=f"rcE{p}")
                for qt in qts:
                    sb_q = pn.tile([128, 128], F32, tag="sbq", name=f"sbq{qt}")
                    nc.vector.tensor_copy(
                        sb_q[:], psy[:, qt * 128:(qt + 1) * 128])
                    nc.vector.reciprocal(rc[:, qt:qt + 1], sb_q[:, 64:65])
                    eng = nc.vector if qt == 3 else nc.gpsimd
                    eng.tensor_scalar_mul(
                        y_sb[qt][:, h_loc * 64:h_loc * 64 + 64],
                        sb_q[:, 0:64], rc[:, qt:qt + 1])
                    nc.sync.dma_start_transpose(
                        yT[1][:, qj * 512 + qt * 128:qj * 512 + (qt + 1) * 128],
                        y_sb[qt][:, 128:256])
                continue
            if p != npair - 1:
                continue
            # last pair of this head: normalize y = psy * (1/l).
            # GPSIMD can't read PSUM: drain psy to SBUF once (DVE), then
            # reciprocal + per-head muls run off SBUF (Pool-legal).
            if qj not in ysb_tiles:
                ysb_tiles[qj] = [
                    pysb.tile([128, 256], BF, tag="ysb", name=f"ysb{qj}_{q}")
                    for q in range(4)]
            y_sb = ysb_tiles[qj]
            sb_y = pn.tile([128, 512], F32, tag="sby")
            nc.vector.tensor_copy(sb_y[:], psy[:])
            rc = pn.tile([128, 4], F32, tag="rc")
            for qt in range(4):
                nc.vector.reciprocal(
                    rc[:, qt:qt + 1], sb_y[:, qt * 128 + 64:qt * 128 + 65])
            for qt in range(4):
                nc.gpsimd.tensor_scalar_mul(
                    y_sb[qt][:, h_loc * 64:h_loc * 64 + 64],
                    sb_y[:, qt * 128:qt * 128 + 64],
                    rc[:, qt:qt + 1])
            if lh == 1:
                # both heads of this pair done: transpose to yT
                for qt in range(4):
                    nc.sync.dma_start_transpose(
                        yT[hp][:, qj * 512 + qt * 128:qj * 512 + (qt + 1) * 128],
                        y_sb[qt][:, hp * 128:(hp + 1) * 128])
                if hp == 1:
                    for mo in range(8):
                        fq.append([10**9, proj(qj, mo), 427])
        # drain leftover filler inside the attention scope, then run proj(3)
        # through a wide PSUM ring (psS/psY banks released) so its 8 blocks
        # stream without ring stalls
        while fq:
            fq.pop(0)[1]()
        inner.close()
        # tail proj: copies split DVE/Act, outputs staged into one tile so a
        # single strided DMA replaces 8 serialized HWDGE generations
        with tc.tile_pool(name="tail", bufs=6, space="PSUM") as ptail:
            ot_mega = po.tile([128, 8, 512], BF, tag="otm", name="ot_mega")
            # qt01 columns of yT(3) finish one pair earlier than qt23 (the
            # epilogue transposes them at p==6), so for 6 of 8 mo blocks the
            # first-half matmuls pre-run during the final exp window
            pps_t = {}
            for mo in range(6):
                pps = ptail.tile([128, 512], F32, tag="tp", name=f"tp{mo}")
                pps_t[mo] = pps
                for kt2 in range(2):
                    nc.tensor.matmul(
                        pps[:, 0:256],
                        wp[:, kt2 * C + mo * 128:kt2 * C + (mo + 1) * 128],
                        yT[kt2][:, 3 * 512:3 * 512 + 256],
                        start=(kt2 == 0), stop=(kt2 == 1),
                        skip_group_check=True)
            for mo in range(8):
                if mo < 6:
                    pps = pps_t[mo]
                    for kt2 in range(2):
                        nc.tensor.matmul(
                            pps[:, 256:512],
                            wp[:, kt2 * C + mo * 128:kt2 * C + (mo + 1) * 128],
                            yT[kt2][:, 3 * 512 + 256:4 * 512],
                            start=False, stop=(kt2 == 1),
                            skip_group_check=True)
                else:
                    pps = ptail.tile([128, 512], F32, tag="tp", name=f"tp{mo}")
                    for kt2 in range(2):
                        nc.tensor.matmul(
                            pps[:],
                            wp[:, kt2 * C + mo * 128:kt2 * C + (mo + 1) * 128],
                            yT[kt2][:, 3 * 512:4 * 512],
                            start=(kt2 == 0), stop=(kt2 == 1))
                if mo % 2 == 1:
                    nc.scalar.activation(
                        ot_mega[:, mo, :], pps[:],
                        mybir.ActivationFunctionType.Copy)
                else:
                    nc.vector.tensor_copy(ot_mega[:, mo, :], pps[:])
                # split the final DMA: the bulk streams out while the last
                # two blocks' copies finish, shortening the terminal chain
                if mo == 5:
                    nc.sync.dma_start(
                        d_out[:, 0:6, 3 * 512:4 * 512], ot_mega[:, 0:6, :])
                elif mo == 6:
                    nc.sync.dma_start(
                        d_out[:, 6, 3 * 512:4 * 512], ot_mega[:, 6, :])
            nc.sync.dma_start(
                d_out[:, 7, 3 * 512:4 * 512], ot_mega[:, 7, :])


def _build(reps=1):
    nc = bacc.Bacc("TRN2", target_bir_lowering=False, debug=False)
    d = (
        nc.dram_tensor("xhi", [128, TJ, CTP, 2, 512], FP8, kind="ExternalInput").ap(),
        nc.dram_tensor("xlo", [128, TJ, CTP, 2, 512], FP8, kind="ExternalInput").ap(),
        nc.dram_tensor("wqh", [128, 4, CTP, 2, 128], FP8, kind="ExternalInput").ap(),
        nc.dram_tensor("wql", [128, 4, CTP, 2, 128], FP8, kind="ExternalInput").ap(),
        nc.dram_tensor("wvh", [128, CTP, 2, VW], FP8, kind="ExternalInput").ap(),
        nc.dram_tensor("wvl", [128, CTP, 2, VW], FP8, kind="ExternalInput").ap(),
        nc.dram_tensor("wp", [128, 2 * C], BF, kind="ExternalInput").ap(),
        nc.dram_tensor("bqk", [128, 4], F32, kind="ExternalInput").ap(),
        nc.dram_tensor("tri", [128, 128], FP8, kind="ExternalInput").ap(),
        nc.dram_tensor("ident", [128, 128], FP8, kind="ExternalInput").ap(),
        nc.dram_tensor("outT", [128, 8, T], BF, kind="ExternalOutput").ap(),
    )
    with tile.TileContext(nc) as tc:
        for rep in range(reps):
            if rep:
                tc.strict_bb_all_engine_barrier()
            _emit(tc, nc, d)
    nc.compile()
    return nc


def _sb(a):
    """[128k, n] -> SBUF layout [128, k, n] (k-tile-major along free dim)."""
    k = a.shape[0] // 128
    return np.ascontiguousarray(
        a.reshape(k, 128, a.shape[1]).transpose(1, 0, 2))


def _f8(a):
    return np.ascontiguousarray(a).astype(NFP8)


def _split8(a):
    """f32 -> (hi, lo) fp8 residual split."""
    hi = np.asarray(a, np.float32).astype(NFP8)
    lo = (np.asarray(a, np.float32) - hi.astype(np.float32)).astype(NFP8)
    return hi, lo


def _wqk_layout(w):
    """[C, 512] -> [128, mo, ctp, 2, 128] (ct pairs interleaved for DR)."""
    s = _sb(w)                                   # [128, ct(8), 512]
    s = s.reshape(128, CTP, 2, 4, 128).transpose(0, 3, 1, 2, 4)
    return np.ascontiguousarray(s)


def _x_layout(xT):
    """[128, ct, T] -> [128, tj, ctp, 2, 512]."""
    s = xT.reshape(128, CTP, 2, TJ, 512).transpose(0, 3, 1, 2, 4)
    return np.ascontiguousarray(s)


def _wv_layout(w):
    """[C, 256] -> [128, ctp, 2, 256]."""
    s = _sb(w)                                   # [128, ct, 256]
    s = s.reshape(128, CTP, 2, VW)
    return np.ascontiguousarray(s)


def _prep_in_maps(inputs):
    x = np.asarray(inputs["x"], np.float32)
    W_attn = np.asarray(inputs["W_attn"], np.float32)
    b_attn = np.asarray(inputs["b_attn"], np.float32)
    W_proj = np.asarray(inputs["W_proj"], np.float32)

    scale = 1.0 / np.sqrt(HD)
    # -80 lower-triangle (mask k>q within the diagonal tile: j < p)
    tri = np.where(np.arange(128)[None, :] < np.arange(128)[:, None],
                   -80.0, 0.0).astype(np.float32)
    ident = np.eye(128, dtype=np.float32)

    in_maps = []
    for c in range(NCORES):
        b, g = divmod(c, 4)
        heads = [4 * g + i for i in range(HPC)]
        xT = _sb(np.ascontiguousarray(x[b].T))              # [128, ct, 2048]
        xh, xl = _split8(xT)
        xh = _x_layout(xh.astype(np.float32)).astype(NFP8)
        xl = _x_layout(xl.astype(np.float32)).astype(NFP8)

        wq = [W_attn[:, h * HD:(h + 1) * HD] * (scale * SQ) for h in heads]
        wk = [W_attn[:, C + h * HD:C + (h + 1) * HD] * SK for h in heads]
        wqk = np.concatenate(
            [wq[0], wq[1], wk[0], wk[1], wq[2], wq[3], wk[2], wk[3]], axis=1)
        wqk_hi, wqk_lo = _split8(wqk)
        wqh = _wqk_layout(wqk_hi.astype(np.float32)).astype(NFP8)
        wql = _wqk_layout(wqk_lo.astype(np.float32)).astype(NFP8)

        wv = np.concatenate(
            [W_attn[:, 2 * C + h * HD:2 * C + (h + 1) * HD] for h in heads],
            axis=1) * SV                                    # [C, 256]
        wv_hi, wv_lo = _split8(wv)
        wvh = _wv_layout(wv_hi.astype(np.float32)).astype(NFP8)
        wvl = _wv_layout(wv_lo.astype(np.float32)).astype(NFP8)

        wp = np.zeros((128, 2 * C), np.float32)
        for kt2 in range(2):
            rows = np.concatenate(
                [W_proj[heads[2 * kt2 + j] * HD:(heads[2 * kt2 + j] + 1) * HD, :]
                 for j in range(2)], axis=0)                # [128, 1024]
            wp[:, kt2 * C:(kt2 + 1) * C] = rows

        bqk = np.zeros((128, 4), np.float32)
        for i2 in range(2):   # head pair
            for j in range(2):
                h = heads[2 * i2 + j]
                bqk[64 * j:64 * j + 64, 2 * i2] = b_attn[h * HD:(h + 1) * HD] * scale
                bqk[64 * j:64 * j + 64, 2 * i2 + 1] = b_attn[C + h * HD:C + (h + 1) * HD]

        in_maps.append({"xhi": xh, "xlo": xl, "wqh": wqh, "wql": wql,
                        "wvh": wvh, "wvl": wvl,
                        "wp": np.ascontiguousarray(wp).astype(ml_dtypes.bfloat16),
                        "bqk": bqk, "tri": _f8(tri), "ident": _f8(ident)})
    return in_maps


def kernel(x, W_attn, b_attn, W_proj, b_proj):
    in_maps = _prep_in_maps(dict(x=x, W_attn=W_attn, b_attn=b_attn,
                                 W_proj=W_proj, b_proj=b_proj))
    if "nc" not in _CACHE:
        _CACHE["nc"] = _build()
    nc = _CACHE["nc"]
    res = run_bass_kernel_spmd(nc, in_maps, core_ids=list(range(NCORES)))

    out = np.zeros((B, T, C), np.float32)
    for c in range(NCORES):
        b = c // 4
        oT = np.asarray(res.results[c]["outT"], np.float32)         # [128, 8*2048]
        oT = oT.reshape(128, 8, T).transpose(1, 0, 2).reshape(C, T)  # [C, T]
        out[b] += oT.T
    # bv was dropped from the device V path: y_true = y_dev + bv, so
    # out_true = out_dev + bv @ W_proj (+ b_proj), both added here.
    bv_full = np.asarray(b_attn, np.float32)[2 * C:3 * C]
    out += (bv_full @ np.asarray(W_proj, np.float32))[None, None, :]
    out += np.asarray(b_proj, np.float32)[None, None, :]
    return out


# revision 19
# speedup vs baseline: 1.0686x; 1.0686x over previous
"""Causal self-attention (B=2,T=2048,C=1024,H=16) on 8 trn2 NeuronCores.

Sharding: core c handles batch b=c//4 and 4 heads (c%4)*4..+4 (tensor-parallel
over heads x data-parallel over batch).

v2: fp8 DoubleRow matmuls where numerics allow (residual-split operands):
  stage A (qk and v): 3-product fp8-DR  x_hi@W_hi + x_hi@W_lo + x_lo@W_hi
    (x split host-side into fp8 hi+lo; W pre-scaled to unit rms and split).
    Per-group pow2 scales undone in the drain (tensor_scalar mult+add).
  scores: bf16 as before (fp8 there breaks the 2e-2 tolerance), with per-ki
    off-trim; causal mask applied by accumulating a -80 triangular tile into
    PSUM via a tiny identity-stationary matmul (PE) instead of es*mask on
    DVE/Pool.
  exp: Act engine, bias -3 (fp8 range headroom), es written as fp8e4m3.
  AV: DoubleRow pairs (es,es) stride-0 stationary x (v_hi,v_lo) moving;
    V drained as fp8 hi+lo split (exact to ~0.1%); l ones-column is a
    constant region of the V tiles (bv folded into b_proj host-side).
  proj: bf16 unchanged.  y normalize/transpose/tail: unchanged.
"""
import sys

sys.path.insert(0, "/opt/trn_rl_repo")

import numpy as np
import ml_dtypes

import concourse.bass as bass
import concourse.mybir as mybir
import concourse.tile as tile
from concourse import bacc
from concourse.bass_utils import run_bass_kernel_spmd

B, T, C, H, HD = 2, 2048, 1024, 16, 64
NCORES = 8
HPC = 4            # heads per core
CT = C // 128      # 8 contraction tiles
CTP = CT // 2      # 4 contraction-tile pairs (DoubleRow)
TJ = T // 512      # 4 q chunks
TT = T // 128      # 16 tok tiles
VW = 256           # V matmul cols per core (4 heads x 64, l-cols separate)
F32 = mybir.dt.float32
BF = mybir.dt.bfloat16
FP8 = mybir.dt.float8e4
EXP = mybir.ActivationFunctionType.Exp
DR = mybir.MatmulPerfMode.DoubleRow
NFP8 = ml_dtypes.float8_e4m3

SQ = 256.0         # wq fp8 pre-scale (alpha*Wq ~ 1/256 rms)
SK = 32.0          # wk fp8 pre-scale
SV = 32.0          # wv fp8 pre-scale
EB = 3.0           # exp bias (es = exp(s-3), max ~137 < fp8 240)

_CACHE = {}


def _emit(tc, nc, d):
    (d_xhi, d_xlo, d_wqh, d_wql, d_wvh, d_wvl, d_wp, d_bqk, d_tri, d_ident,
     d_out) = d
    from contextlib import ExitStack
    with tc.tile_pool(name="const", bufs=1) as pc, \
         tc.tile_pool(name="qk", bufs=1) as pqk, \
         tc.tile_pool(name="vv", bufs=1) as pvv, \
         tc.tile_pool(name="yt", bufs=1) as pyt, \
         tc.tile_pool(name="w_in", bufs=1) as pw, \
         tc.tile_pool(name="x_in", bufs=1) as px, \
         tc.tile_pool(name="fill", bufs=2, space="PSUM") as pfill, \
         tc.tile_pool(name="ex", bufs=4) as pex, \
         tc.tile_pool(name="nrm", bufs=4) as pn, \
         tc.tile_pool(name="ysb", bufs=8) as pysb, \
         tc.tile_pool(name="po", bufs=4) as po:
        inner = ExitStack()
        psS = inner.enter_context(tc.tile_pool(name="psS", bufs=2, space="PSUM"))
        psY = inner.enter_context(tc.tile_pool(name="psY", bufs=2, space="PSUM"))
        bqk = pc.tile([128, 4], F32, tag="bqk")
        tri = pc.tile([128, 128], FP8, tag="tri")
        ident = pc.tile([128, 128], FP8, tag="ident")
        negeb = pc.tile([128, 1], F32, tag="negeb")
        warm = pc.tile([128, 512], BF, tag="warm")
        nc.gpsimd.memset(negeb[:], -EB)
        nc.gpsimd.memset(warm[:], 0.0)
        # preload the Exp activation table during the input-DMA wait so the
        # first real exp doesn't pay the 1.3us table load
        wes = pex.tile([128, 2, 512], FP8, tag="es", name="warmes")
        nc.scalar.activation(wes[0:1, 0, 0:1], negeb[0:1, :], EXP, bias=0.0)

        qkT = [pqk.tile([128, T], BF, tag=f"qk{i}", name=f"qkT{i}") for i in range(4)]
        # V tiles: [128 tok, 2(hi/lo), 4 heads, 65]; col 64 of each head block
        # is the constant l-column (hi=1, lo=0), set once below.
        V = [pvv.tile([128, 2, HPC, 65], FP8, tag=f"v{i}", name=f"V{i}")
             for i in range(TT)]
        yT = [pyt.tile([128, T], BF, tag=f"y{i}", name=f"yT{i}") for i in range(2)]
        wqh = pw.tile([128, 4, CTP, 2, 128], FP8, tag="wqh")
        wql = pw.tile([128, 4, CTP, 2, 128], FP8, tag="wql")
        wvh = pw.tile([128, CTP, 2, VW], FP8, tag="wvh")
        wvl = pw.tile([128, CTP, 2, VW], FP8, tag="wvl")
        wp = pc.tile([128, 2 * C], BF, tag="wp")
        xhi = px.tile([128, TJ, CTP, 2, 512], FP8, tag="xhi")
        xlo = px.tile([128, TJ, CTP, 2, 512], FP8, tag="xlo")

        # input DMAs, ordered so the first a_qk/a_v blocks unblock earliest
        nc.sync.dma_start(wqh[:, 0], d_wqh[:, 0])
        nc.sync.dma_start(xhi[:, 0], d_xhi[:, 0])
        nc.sync.dma_start(wql[:, 0], d_wql[:, 0])
        nc.sync.dma_start(xlo[:, 0], d_xlo[:, 0])
        for mo in range(1, 4):
            nc.sync.dma_start(wqh[:, mo], d_wqh[:, mo])
            nc.sync.dma_start(wql[:, mo], d_wql[:, mo])
        nc.sync.dma_start(bqk[:], d_bqk)
        nc.sync.dma_start(wvh[:], d_wvh)
        nc.sync.dma_start(wvl[:], d_wvl)
        nc.sync.dma_start(tri[:], d_tri)
        nc.sync.dma_start(ident[:], d_ident)
        for tj in range(1, TJ):
            nc.sync.dma_start(xhi[:, tj], d_xhi[:, tj])
            nc.sync.dma_start(xlo[:, tj], d_xlo[:, tj])
        nc.sync.dma_start(wp[:], d_wp)

        # constant l-columns of the V tiles (hi=1 -> l = sum es; lo=0)
        for tt in range(TT):
            nc.gpsimd.memset(V[tt][:, 0, :, 64:65], 1.0)
            nc.gpsimd.memset(V[tt][:, 1, :, 64:65], 0.0)

        # PE p-state warmup during the input-DMA wait (results unused)
        ws = psS.tile([128, 2, 512], F32, tag="s", name="warms")
        for i in range(8):
            nc.tensor.matmul(ws[0:64, 0, :], warm[:, :64], warm[:, :],
                             start=True, stop=True)

        # ---------------- stage A blocks (emitted via filler queue) -------
        # 3-product fp8 DoubleRow: xh@Wh + xh@Wl + xl@Wh (x/W host-split).
        def a_qk(tj, mo):
            sc_ = (1.0 / SQ) if mo % 2 == 0 else (1.0 / SK)

            def emit():
                ps = pfill.tile([128, 512], F32, tag="fill", name=f"psqk{tj}_{mo}")
                n = 0
                for wt, xt in ((wqh, xhi), (wql, xhi), (wqh, xlo)):
                    for cp in range(CTP):
                        nc.tensor.matmul(
                            ps[:], wt[:, mo, cp], xt[:, tj, cp],
                            start=(n == 0), stop=(n == 11), perf_mode=DR)
                        n += 1
                nc.vector.tensor_scalar(
                    qkT[mo][:, tj * 512:(tj + 1) * 512], ps[:],
                    sc_, bqk[:, mo:mo + 1],
                    mybir.AluOpType.mult, mybir.AluOpType.add)
            return emit

        def a_v(tt):
            tj, ti = divmod(tt, 4)

            def emit():
                psv = pfill.tile([128, 512], F32, tag="fill", name=f"psv{tt}")
                n = 0
                for wt, xt in ((wvh, xhi), (wvl, xhi), (wvh, xlo)):
                    for cp in range(CTP):
                        nc.tensor.matmul(
                            psv[:, :VW],
                            xt[:, tj, cp, :, ti * 128:(ti + 1) * 128],
                            wt[:, cp],
                            start=(n == 0), stop=(n == 11), perf_mode=DR)
                        n += 1
                # split drain: hi = fp8(psv/SV), lo = fp8(psv/SV - hi)
                nc.vector.tensor_scalar(
                    V[tt][:, 0, :, 0:64], psv[:, :VW], 1.0 / SV, None,
                    mybir.AluOpType.mult)
                nc.vector.scalar_tensor_tensor(
                    V[tt][:, 1, :, 0:64], psv[:, :VW], 1.0 / SV,
                    V[tt][:, 0, :, 0:64],
                    mybir.AluOpType.mult, mybir.AluOpType.subtract)
            return emit

        def proj(qj, mo, pool=None, ptag="fill", act_copy=False):
            def emit():
                pps = (pool or pfill).tile([128, 512], F32, tag=ptag,
                                           name=f"pps{qj}_{mo}")
                for kt2 in range(2):
                    nc.tensor.matmul(
                        pps[:],
                        wp[:, kt2 * C + mo * 128:kt2 * C + (mo + 1) * 128],
                        yT[kt2][:, qj * 512:(qj + 1) * 512],
                        start=(kt2 == 0), stop=(kt2 == 1))
                ot = po.tile([128, 512], BF, tag="ot")
                if act_copy:
                    nc.scalar.activation(
                        ot[:], pps[:], mybir.ActivationFunctionType.Copy)
                else:
                    nc.vector.tensor_copy(ot[:], pps[:])
                nc.sync.dma_start(
                    d_out[:, mo, qj * 512:(qj + 1) * 512], ot[:])
            return emit

        # stage A for tj=0 runs up front; the rest interleaves into attention
        for mo in range(4):
            a_qk(0, mo)()
        for tt in range(4):
            a_v(tt)()

        # -------- attention: one global pair pipeline across sections -----
        units = []
        for qj, hp in [(0, 0), (0, 1), (1, 0), (1, 1),
                       (2, 0), (3, 0), (2, 1), (3, 1)]:
            for lh in range(2):
                for p in range(2 * qj + 2):
                    units.append((qj, hp, lh, p))

        fq = []
        for tj in range(1, TJ):
            for mo in range(4):
                fq.append(["A", tj, a_qk(tj, mo), 1280])
            for tt in range(4 * tj, 4 * tj + 4):
                fq.append(["A", tj, a_v(tt), 640])

        # debt-based pacing: filler is emitted only while the PE work emitted
        # so far trails the (scaled) Act work emitted so far.  The 1.06
        # factor spreads the PE surplus evenly instead of exhausting the
        # filler before the late Act-bound windows.
        clock = {"pe": 0.0, "act": 0.0}

        def drain_filler(cur_qj, need_tj=None, budget=False, force=0):
            i = 0
            n = 0
            while i < len(fq):
                kind, idx, fn, cost = fq[i][:4]
                forced = need_tj is not None and kind == "A" and idx <= need_tj
                if not forced:
                    if n >= force and (
                            not budget
                            or clock["pe"] >= clock["act"] * 1.06 - 300):
                        break
                    if kind == "A" and idx > cur_qj + 1:
                        i += 1
                        continue
                fn()
                clock["pe"] += cost
                fq.pop(i)
                if not forced:
                    n += 1

        ysb_tiles = {}
        state = {}

        def sc(u):
            """Scores for unit u: per-ki trimmed matmul + PE causal mask."""
            qj, hp, lh, p = u
            s = psS.tile([128, 2, 512], F32, tag="s")
            es = pex.tile([128, 2, 512], FP8, tag="es")
            for ki in range(2):
                kt = 2 * p + ki
                r = kt - 4 * qj
                off = 128 * r if r >= 0 else 0
                nc.tensor.matmul(
                    s[:, ki, off:512],
                    qkT[2 * hp + 1][64 * lh:64 * lh + 64, kt * 128:(kt + 1) * 128],
                    qkT[2 * hp][64 * lh:64 * lh + 64,
                                qj * 512 + off:(qj + 1) * 512],
                    start=True, stop=(r < 0))
                if r >= 0:
                    # causal mask: accumulate -80 lower-triangle into the
                    # diagonal tile (exp then yields ~0; no DVE/Pool mask mul)
                    nc.tensor.matmul(
                        s[:, ki, off:off + 128], ident[:], tri[:],
                        start=False, stop=True, skip_group_check=True)
                clock["pe"] += (512 - off) * 0.4167 + (53.3 if r >= 0 else 0)
            return s, es

        # distance to the next qj-crossing, to pre-spread the A-barrier
        nxt = [len(units)] * len(units)
        for i in range(len(units) - 2, -1, -1):
            nxt[i] = i + 1 if units[i + 1][0] != units[i][0] else nxt[i + 1]

        pend = {}
        for i, u in enumerate(units):
            qj, hp, lh, p = u
            npair = 2 * qj + 2
            h_loc = 2 * hp + lh
            if i == 0:
                drain_filler(qj, need_tj=qj)
                pend[0] = sc(u)
            if i + 1 < len(units):
                nqj = units[i + 1][0]
                if nqj > qj:
                    drain_filler(qj, need_tj=nqj)
                pend[i + 1] = sc(units[i + 1])
            s, es = pend.pop(i)
            off = 256 if p == 2 * qj + 1 else 0
            nc.scalar.activation(
                es[:, :, off:512], s[:, :, off:512], EXP, bias=negeb[:])
            clock["act"] += (1024 - 2 * off) * 0.833 + 219
            near_cross = (nxt[i] - i <= 6 and nxt[i] < len(units)
                          and any(e[0] == "A" and e[1] <= units[nxt[i]][0]
                                  for e in fq))
            drain_filler(qj, budget=True,
                         force=1 if (near_cross or (qj, hp) == (3, 1)) else 0)
            # AV (transposed, DoubleRow): psy[qt] += es_kt(qt-slice) @ (vhi|vlo)
            key = (qj, hp, lh)
            if key not in state:
                state[key] = [psY.tile([128, 512], F32, tag="psy",
                                       name=f"psy{qj}_{hp}_{lh}"), True]
            psy, first_mm = state[key]
            for qt in range(4):
                for ki in range(2):
                    kt = 2 * p + ki
                    if kt > 4 * qj + qt:
                        continue
                    es_pair = es[:, ki, qt * 128:(qt + 1) * 128] \
                        .unsqueeze(1).broadcast_to([128, 2, 128])
                    nc.tensor.matmul(
                        psy[:, qt * 128:qt * 128 + 65],
                        es_pair,
                        V[kt][:, :, h_loc, :],
                        start=first_mm,
                        stop=(kt == 4 * qj + qt),
                        perf_mode=DR,
                        skip_group_check=True)
                    first_mm = False
                    clock["pe"] += 13.5
            state[key][1] = first_mm
            if (qj, hp, lh) == (3, 1, 1) and p >= 6:
                # epilogue fast-path: normalize + transpose each qt as soon
                # as its AV accumulation stops so only qt2/qt3 trail the
                # final exp
                qts = (0, 1) if p == 6 else (2, 3)
                y_sb = ysb_tiles[qj]
                rc = pn.tile([128, 4], F32, tag="rc", name=f"rcE{p}")
                for qt in qts:
                    sb_q = pn.tile([128, 128], F32, tag="sbq", name=f"sbq{qt}")
                    nc.vector.tensor_copy(
                        sb_q[:], psy[:, qt * 128:(qt + 1) * 128])
                    nc.vector.reciprocal(rc[:, qt:qt + 1], sb_q[:, 64:65])
                    eng = nc.vector if qt == 3 else nc.gpsimd
                    eng.tensor_scalar_mul(
                        y_sb[qt][:, h_loc * 64:h_loc * 64 + 64],
                        sb_q[:, 0:64], rc[:, qt:qt + 1])
                    nc.sync.dma_start_transpose(
                        yT[1][:, qj * 512 + qt * 128:qj * 512 + (qt + 1) * 128],
                        y_sb[qt][:, 128:256])
                continue
            if p != npair - 1:
                continue
            # last pair of this head: normalize y = psy * (1/l).
            # GPSIMD can't read PSUM: drain psy to SBUF once (DVE), then
            # reciprocal + per-head muls run off SBUF (Pool-legal).
            if qj not in ysb_tiles:
                ysb_tiles[qj] = [
                    pysb.tile([128, 256], BF, tag="ysb", name=f"ysb{qj}_{q}")
                    for q in range(4)]
            y_sb = ysb_tiles[qj]
            sb_y = pn.tile([128, 512], F32, tag="sby")
            nc.vector.tensor_copy(sb_y[:], psy[:])
            rc = pn.tile([128, 4], F32, tag="rc")
            for qt in range(4):
                nc.vector.reciprocal(
                    rc[:, qt:qt + 1], sb_y[:, qt * 128 + 64:qt * 128 + 65])
            for qt in range(4):
                nc.gpsimd.tensor_scalar_mul(
                    y_sb[qt][:, h_loc * 64:h_loc * 64 + 64],
                    sb_y[:, qt * 128:qt * 128 + 64],
                    rc[:, qt:qt + 1])
            if lh == 1:
                # both heads of this pair done: transpose to yT
                for qt in range(4):
                    nc.sync.dma_start_transpose(
                        yT[hp][:, qj * 512 + qt * 128:qj * 512 + (qt + 1) * 128],
                        y_sb[qt][:, hp * 128:(hp + 1) * 128])
                if hp == 1:
                    for mo in range(8):
                        fq.append(["P", qj, proj(qj, mo), 427, mo])
        # drain leftover filler inside the attention scope, then run proj(3)
        # through a wide PSUM ring (psS/psY banks released) so its 8 blocks
        # stream without ring stalls
        while fq:
            fq.pop(0)[2]()
        inner.close()
        # tail proj: copies split DVE/Act, outputs staged into one tile so a
        # single strided DMA replaces 8 serialized HWDGE generations
        with tc.tile_pool(name="tail", bufs=6, space="PSUM") as ptail:
            ot_mega = po.tile([128, 8, 512], BF, tag="otm", name="ot_mega")
            # qt01 columns of yT(3) finish one pair earlier than qt23 (the
            # epilogue transposes them at p==6), so for 6 of 8 mo blocks the
            # first-half matmuls pre-run during the final exp window
            pps_t = {}
            for mo in range(6):
                pps = ptail.tile([128, 512], F32, tag="tp", name=f"tp{mo}")
                pps_t[mo] = pps
                for kt2 in range(2):
                    nc.tensor.matmul(
                        pps[:, 0:256],
                        wp[:, kt2 * C + mo * 128:kt2 * C + (mo + 1) * 128],
                        yT[kt2][:, 3 * 512:3 * 512 + 256],
                        start=(kt2 == 0), stop=(kt2 == 1),
                        skip_group_check=True)
            for mo in range(8):
                if mo < 6:
                    pps = pps_t[mo]
                    for kt2 in range(2):
                        nc.tensor.matmul(
                            pps[:, 256:512],
                            wp[:, kt2 * C + mo * 128:kt2 * C + (mo + 1) * 128],
                            yT[kt2][:, 3 * 512 + 256:4 * 512],
                            start=False, stop=(kt2 == 1),
                            skip_group_check=True)
                else:
                    pps = ptail.tile([128, 512], F32, tag="tp", name=f"tp{mo}")
                    for kt2 in range(2):
                        nc.tensor.matmul(
                            pps[:],
                            wp[:, kt2 * C + mo * 128:kt2 * C + (mo + 1) * 128],
                            yT[kt2][:, 3 * 512:4 * 512],
                            start=(kt2 == 0), stop=(kt2 == 1))
                if mo % 2 == 1:
                    nc.scalar.activation(
                        ot_mega[:, mo, :], pps[:],
                        mybir.ActivationFunctionType.Copy)
                else:
                    nc.vector.tensor_copy(ot_mega[:, mo, :], pps[:])
                # split the final DMA: the bulk streams out while the last
                # two blocks' copies finish, shortening the terminal chain
                if mo == 5:
                    nc.sync.dma_start(
                        d_out[:, 0:6, 3 * 512:4 * 512], ot_mega[:, 0:6, :])
                elif mo == 6:
                    nc.sync.dma_start(
                        d_out[:, 6, 3 * 512:4 * 512], ot_mega[:, 6, :])
            nc.sync.dma_start(
                d_out[:, 7, 3 * 512:4 * 512], ot_mega[:, 7, :])


def _build(reps=1):
    nc = bacc.Bacc("TRN2", target_bir_lowering=False, debug=False)
    d = (
        nc.dram_tensor("xhi", [128, TJ, CTP, 2, 512], FP8, kind="ExternalInput").ap(),
        nc.dram_tensor("xlo", [128, TJ, CTP, 2, 512], FP8, kind="ExternalInput").ap(),
        nc.dram_tensor("wqh", [128, 4, CTP, 2, 128], FP8, kind="ExternalInput").ap(),
        nc.dram_tensor("wql", [128, 4, CTP, 2, 128], FP8, kind="ExternalInput").ap(),
        nc.dram_tensor("wvh", [128, CTP, 2, VW], FP8, kind="ExternalInput").ap(),
        nc.dram_tensor("wvl", [128, CTP, 2, VW], FP8, kind="ExternalInput").ap(),
        nc.dram_tensor("wp", [128, 2 * C], BF, kind="ExternalInput").ap(),
        nc.dram_tensor("bqk", [128, 4], F32, kind="ExternalInput").ap(),
        nc.dram_tensor("tri", [128, 128], FP8, kind="ExternalInput").ap(),
        nc.dram_tensor("ident", [128, 128], FP8, kind="ExternalInput").ap(),
        nc.dram_tensor("outT", [128, 8, T], BF, kind="ExternalOutput").ap(),
    )
    with tile.TileContext(nc) as tc:
        for rep in range(reps):
            if rep:
                tc.strict_bb_all_engine_barrier()
            _emit(tc, nc, d)
    nc.compile()
    return nc


def _sb(a):
    """[128k, n] -> SBUF layout [128, k, n] (k-tile-major along free dim)."""
    k = a.shape[0] // 128
    return np.ascontiguousarray(
        a.reshape(k, 128, a.shape[1]).transpose(1, 0, 2))


def _f8(a):
    return np.ascontiguousarray(a).astype(NFP8)


def _split8(a):
    """f32 -> (hi, lo) fp8 residual split."""
    hi = np.asarray(a, np.float32).astype(NFP8)
    lo = (np.asarray(a, np.float32) - hi.astype(np.float32)).astype(NFP8)
    return hi, lo


def _wqk_layout(w):
    """[C, 512] -> [128, mo, ctp, 2, 128] (ct pairs interleaved for DR)."""
    s = _sb(w)                                   # [128, ct(8), 512]
    s = s.reshape(128, CTP, 2, 4, 128).transpose(0, 3, 1, 2, 4)
    return np.ascontiguousarray(s)


def _x_layout(xT):
    """[128, ct, T] -> [128, tj, ctp, 2, 512]."""
    s = xT.reshape(128, CTP, 2, TJ, 512).transpose(0, 3, 1, 2, 4)
    return np.ascontiguousarray(s)


def _wv_layout(w):
    """[C, 256] -> [128, ctp, 2, 256]."""
    s = _sb(w)                                   # [128, ct, 256]
    s = s.reshape(128, CTP, 2, VW)
    return np.ascontiguousarray(s)


def _prep_in_maps(inputs):
    x = np.asarray(inputs["x"], np.float32)
    W_attn = np.asarray(inputs["W_attn"], np.float32)
    b_attn = np.asarray(inputs["b_attn"], np.float32)
    W_proj = np.asarray(inputs["W_proj"], np.float32)

    scale = 1.0 / np.sqrt(HD)
    # -80 lower-triangle (mask k>q within the diagonal tile: j < p)
    tri = np.where(np.arange(128)[None, :] < np.arange(128)[:, None],
                   -80.0, 0.0).astype(np.float32)
    ident = np.eye(128, dtype=np.float32)

    in_maps = []
    for c in range(NCORES):
        b, g = divmod(c, 4)
        heads = [4 * g + i for i in range(HPC)]
        xT = _sb(np.ascontiguousarray(x[b].T))              # [128, ct, 2048]
        xh, xl = _split8(xT)
        xh = _x_layout(xh.astype(np.float32)).astype(NFP8)
        xl = _x_layout(xl.astype(np.float32)).astype(NFP8)

        wq = [W_attn[:, h * HD:(h + 1) * HD] * (scale * SQ) for h in heads]
        wk = [W_attn[:, C + h * HD:C + (h + 1) * HD] * SK for h in heads]
        wqk = np.concatenate(
            [wq[0], wq[1], wk[0], wk[1], wq[2], wq[3], wk[2], wk[3]], axis=1)
        wqk_hi, wqk_lo = _split8(wqk)
        wqh = _wqk_layout(wqk_hi.astype(np.float32)).astype(NFP8)
        wql = _wqk_layout(wqk_lo.astype(np.float32)).astype(NFP8)

        wv = np.concatenate(
            [W_attn[:, 2 * C + h * HD:2 * C + (h + 1) * HD] for h in heads],
            axis=1) * SV                                    # [C, 256]
        wv_hi, wv_lo = _split8(wv)
        wvh = _wv_layout(wv_hi.astype(np.float32)).astype(NFP8)
        wvl = _wv_layout(wv_lo.astype(np.float32)).astype(NFP8)

        wp = np.zeros((128, 2 * C), np.float32)
        for kt2 in range(2):
            rows = np.concatenate(
                [W_proj[heads[2 * kt2 + j] * HD:(heads[2 * kt2 + j] + 1) * HD, :]
                 for j in range(2)], axis=0)                # [128, 1024]
            wp[:, kt2 * C:(kt2 + 1) * C] = rows

        bqk = np.zeros((128, 4), np.float32)
        for i2 in range(2):   # head pair
            for j in range(2):
                h = heads[2 * i2 + j]
                bqk[64 * j:64 * j + 64, 2 * i2] = b_attn[h * HD:(h + 1) * HD] * scale
                bqk[64 * j:64 * j + 64, 2 * i2 + 1] = b_attn[C + h * HD:C + (h + 1) * HD]

        in_maps.append({"xhi": xh, "xlo": xl, "wqh": wqh, "wql": wql,
                        "wvh": wvh, "wvl": wvl,
                        "wp": np.ascontiguousarray(wp).astype(ml_dtypes.bfloat16),
                        "bqk": bqk, "tri": _f8(tri), "ident": _f8(ident)})
    return in_maps


def kernel(x, W_attn, b_attn, W_proj, b_proj):
    in_maps = _prep_in_maps(dict(x=x, W_attn=W_attn, b_attn=b_attn,
                                 W_proj=W_proj, b_proj=b_proj))
    if "nc" not in _CACHE:
        _CACHE["nc"] = _build()
    nc = _CACHE["nc"]
    res = run_bass_kernel_spmd(nc, in_maps, core_ids=list(range(NCORES)))

    out = np.zeros((B, T, C), np.float32)
    for c in range(NCORES):
        b = c // 4
        oT = np.asarray(res.results[c]["outT"], np.float32)         # [128, 8*2048]
        oT = oT.reshape(128, 8, T).transpose(1, 0, 2).reshape(C, T)  # [C, T]
        out[b] += oT.T
    # bv was dropped from the device V path: y_true = y_dev + bv, so
    # out_true = out_dev + bv @ W_proj (+ b_proj), both added here.
    bv_full = np.asarray(b_attn, np.float32)[2 * C:3 * C]
    out += (bv_full @ np.asarray(W_proj, np.float32))[None, None, :]
    out += np.asarray(b_proj, np.float32)[None, None, :]
    return out


# revision 24
# speedup vs baseline: 1.0763x; 1.0072x over previous
"""Causal self-attention (B=2,T=2048,C=1024,H=16) on 8 trn2 NeuronCores.

Sharding: core c handles batch b=c//4 and 4 heads (c%4)*4..+4 (tensor-parallel
over heads x data-parallel over batch).

v2: fp8 DoubleRow matmuls where numerics allow (residual-split operands):
  stage A (qk and v): 3-product fp8-DR  x_hi@W_hi + x_hi@W_lo + x_lo@W_hi
    (x split host-side into fp8 hi+lo; W pre-scaled to unit rms and split).
    Per-group pow2 scales undone in the drain (tensor_scalar mult+add).
  scores: bf16 as before (fp8 there breaks the 2e-2 tolerance), with per-ki
    off-trim; causal mask applied by accumulating a -80 triangular tile into
    PSUM via a tiny identity-stationary matmul (PE) instead of es*mask on
    DVE/Pool.
  exp: Act engine, bias -3 (fp8 range headroom), es written as fp8e4m3.
  AV: DoubleRow pairs (es,es) stride-0 stationary x (v_hi,v_lo) moving;
    V drained as fp8 hi+lo split (exact to ~0.1%); l ones-column is a
    constant region of the V tiles (bv folded into b_proj host-side).
  proj: bf16 unchanged.  y normalize/transpose/tail: unchanged.
"""
import sys

sys.path.insert(0, "/opt/trn_rl_repo")

import numpy as np
import ml_dtypes

import concourse.bass as bass
import concourse.mybir as mybir
import concourse.tile as tile
from concourse import bacc
from concourse.bass_utils import run_bass_kernel_spmd

B, T, C, H, HD = 2, 2048, 1024, 16, 64
NCORES = 8
HPC = 4            # heads per core
CT = C // 128      # 8 contraction tiles
CTP = CT // 2      # 4 contraction-tile pairs (DoubleRow)
TJ = T // 512      # 4 q chunks
TT = T // 128      # 16 tok tiles
VW = 256           # V matmul cols per core (4 heads x 64, l-cols separate)
F32 = mybir.dt.float32
BF = mybir.dt.bfloat16
FP8 = mybir.dt.float8e4
EXP = mybir.ActivationFunctionType.Exp
DR = mybir.MatmulPerfMode.DoubleRow
NFP8 = ml_dtypes.float8_e4m3

SQ = 256.0         # wq fp8 pre-scale (alpha*Wq ~ 1/256 rms)
SK = 32.0          # wk fp8 pre-scale
SV = 32.0          # wv fp8 pre-scale
EB = 3.0           # exp bias (es = exp(s-3), max ~137 < fp8 240)

_CACHE = {}


def _emit(tc, nc, d):
    (d_xhi, d_xlo, d_wqh, d_wql, d_wvh, d_wvl, d_wp, d_bqk, d_tri, d_ident,
     d_out) = d
    from contextlib import ExitStack
    with tc.tile_pool(name="const", bufs=1) as pc, \
         tc.tile_pool(name="qk", bufs=1) as pqk, \
         tc.tile_pool(name="vv", bufs=1) as pvv, \
         tc.tile_pool(name="yt", bufs=1) as pyt, \
         tc.tile_pool(name="w_in", bufs=1) as pw, \
         tc.tile_pool(name="x_in", bufs=1) as px, \
         tc.tile_pool(name="fill", bufs=2, space="PSUM") as pfill, \
         tc.tile_pool(name="ex", bufs=4) as pex, \
         tc.tile_pool(name="nrm", bufs=4) as pn, \
         tc.tile_pool(name="ysb", bufs=8) as pysb, \
         tc.tile_pool(name="po", bufs=4) as po:
        inner = ExitStack()
        psS = inner.enter_context(tc.tile_pool(name="psS", bufs=2, space="PSUM"))
        psY = inner.enter_context(tc.tile_pool(name="psY", bufs=2, space="PSUM"))
        bqk = pc.tile([128, 4], F32, tag="bqk")
        tri = pc.tile([128, 128], FP8, tag="tri")
        ident = pc.tile([128, 128], FP8, tag="ident")
        negeb = pc.tile([128, 1], F32, tag="negeb")
        warm = pc.tile([128, 512], BF, tag="warm")
        nc.gpsimd.memset(negeb[:], -EB)
        nc.gpsimd.memset(warm[:], 0.0)
        # preload the Exp activation table during the input-DMA wait so the
        # first real exp doesn't pay the 1.3us table load
        wes = pex.tile([128, 2, 512], FP8, tag="es", name="warmes")
        nc.scalar.activation(wes[0:1, 0, 0:1], negeb[0:1, :], EXP, bias=0.0)

        qkT = [pqk.tile([128, T], BF, tag=f"qk{i}", name=f"qkT{i}") for i in range(4)]
        # V tiles: [128 tok, 2(hi/lo), 4 heads, 65]; col 64 of each head block
        # is the constant l-column (hi=1, lo=0), set once below.
        V = [pvv.tile([128, 2, HPC, 65], FP8, tag=f"v{i}", name=f"V{i}")
             for i in range(TT)]
        yT = [pyt.tile([128, T], BF, tag=f"y{i}", name=f"yT{i}") for i in range(2)]
        wqh = pw.tile([128, 4, CTP, 2, 128], FP8, tag="wqh")
        wql = pw.tile([128, 4, CTP, 2, 128], FP8, tag="wql")
        wvh = pw.tile([128, CTP, 2, VW], FP8, tag="wvh")
        wvl = pw.tile([128, CTP, 2, VW], FP8, tag="wvl")
        wp = pc.tile([128, 2 * C], BF, tag="wp")
        xhi = px.tile([128, TJ, CTP, 2, 512], FP8, tag="xhi")
        xlo = px.tile([128, TJ, CTP, 2, 512], FP8, tag="xlo")

        # input DMAs, ordered so the first a_qk/a_v blocks unblock earliest
        nc.sync.dma_start(wqh[:, 0], d_wqh[:, 0])
        nc.sync.dma_start(xhi[:, 0], d_xhi[:, 0])
        nc.sync.dma_start(wql[:, 0], d_wql[:, 0])
        nc.sync.dma_start(xlo[:, 0], d_xlo[:, 0])
        for mo in range(1, 4):
            nc.sync.dma_start(wqh[:, mo], d_wqh[:, mo])
            nc.sync.dma_start(wql[:, mo], d_wql[:, mo])
        nc.sync.dma_start(bqk[:], d_bqk)
        nc.sync.dma_start(wvh[:], d_wvh)
        nc.sync.dma_start(wvl[:], d_wvl)
        nc.sync.dma_start(tri[:], d_tri)
        nc.sync.dma_start(ident[:], d_ident)
        for tj in range(1, TJ):
            nc.sync.dma_start(xhi[:, tj], d_xhi[:, tj])
            nc.sync.dma_start(xlo[:, tj], d_xlo[:, tj])
        nc.sync.dma_start(wp[:], d_wp)

        # constant l-columns of the V tiles (hi=1 -> l = sum es; lo=0)
        for tt in range(TT):
            nc.gpsimd.memset(V[tt][:, 0, :, 64:65], 1.0)
            nc.gpsimd.memset(V[tt][:, 1, :, 64:65], 0.0)

        # PE p-state warmup during the input-DMA wait (results unused)
        ws = psS.tile([128, 2, 512], F32, tag="s", name="warms")
        for i in range(8):
            nc.tensor.matmul(ws[0:64, 0, :], warm[:, :64], warm[:, :],
                             start=True, stop=True)

        # ---------------- stage A blocks (emitted via filler queue) -------
        # 3-product fp8 DoubleRow: xh@Wh + xh@Wl + xl@Wh (x/W host-split).
        def a_qk(tj, mo):
            sc_ = (1.0 / SQ) if mo % 2 == 0 else (1.0 / SK)

            def emit():
                ps = pfill.tile([128, 512], F32, tag="fill", name=f"psqk{tj}_{mo}")
                n = 0
                for wt, xt in ((wqh, xhi), (wql, xhi), (wqh, xlo)):
                    for cp in range(CTP):
                        nc.tensor.matmul(
                            ps[:], wt[:, mo, cp], xt[:, tj, cp],
                            start=(n == 0), stop=(n == 11), perf_mode=DR)
                        n += 1
                nc.vector.tensor_scalar(
                    qkT[mo][:, tj * 512:(tj + 1) * 512], ps[:],
                    sc_, bqk[:, mo:mo + 1],
                    mybir.AluOpType.mult, mybir.AluOpType.add)
            return emit

        def a_v(tt):
            tj, ti = divmod(tt, 4)

            def emit():
                psv = pfill.tile([128, 512], F32, tag="fill", name=f"psv{tt}")
                n = 0
                for wt, xt in ((wvh, xhi), (wvl, xhi), (wvh, xlo)):
                    for cp in range(CTP):
                        nc.tensor.matmul(
                            psv[:, :VW],
                            xt[:, tj, cp, :, ti * 128:(ti + 1) * 128],
                            wt[:, cp],
                            start=(n == 0), stop=(n == 11), perf_mode=DR)
                        n += 1
                # split drain: hi = fp8(psv/SV), lo = fp8(psv/SV - hi)
                nc.vector.tensor_scalar(
                    V[tt][:, 0, :, 0:64], psv[:, :VW], 1.0 / SV, None,
                    mybir.AluOpType.mult)
                nc.vector.scalar_tensor_tensor(
                    V[tt][:, 1, :, 0:64], psv[:, :VW], 1.0 / SV,
                    V[tt][:, 0, :, 0:64],
                    mybir.AluOpType.mult, mybir.AluOpType.subtract)
            return emit

        def proj(qj, mo, pool=None, ptag="fill", act_copy=False):
            def emit():
                pps = (pool or pfill).tile([128, 512], F32, tag=ptag,
                                           name=f"pps{qj}_{mo}")
                for kt2 in range(2):
                    nc.tensor.matmul(
                        pps[:],
                        wp[:, kt2 * C + mo * 128:kt2 * C + (mo + 1) * 128],
                        yT[kt2][:, qj * 512:(qj + 1) * 512],
                        start=(kt2 == 0), stop=(kt2 == 1))
                ot = po.tile([128, 512], BF, tag="ot")
                if act_copy:
                    nc.scalar.activation(
                        ot[:], pps[:], mybir.ActivationFunctionType.Copy)
                else:
                    nc.vector.tensor_copy(ot[:], pps[:])
                nc.sync.dma_start(
                    d_out[:, mo, qj * 512:(qj + 1) * 512], ot[:])
            return emit

        # only the blocks unit 0 needs run before its scores; the rest of
        # tj=0 joins the filler queue (deadline-forced like everything else)
        a_qk(0, 0)()
        a_qk(0, 1)()

        # -------- attention: one global pair pipeline across sections -----
        units = []
        for qj, hp in [(0, 0), (0, 1), (1, 0), (1, 1),
                       (2, 0), (3, 0), (2, 1), (3, 1)]:
            for lh in range(2):
                for p in range(2 * qj + 2):
                    units.append((qj, hp, lh, p))

        # deadline[block] = first unit index whose emission needs it
        dl_qk = {}
        dl_v = {}
        for i, (qj, hp, lh, p) in enumerate(units):
            for mo in (2 * hp, 2 * hp + 1):
                dl_qk.setdefault((qj, mo), i)
            for ki in range(2):
                kt = 2 * p + ki
                if kt <= 4 * qj + 3:
                    dl_v.setdefault((qj, kt), i)

        fq = []
        for tt in range(4):
            fq.append(["A", 0, a_v(tt), 640, dl_v[(0, tt)]])
        for mo in (2, 3):
            fq.append(["A", 0, a_qk(0, mo), 1280, dl_qk[(0, mo)]])
        for tj in range(1, TJ):
            for mo in range(4):
                fq.append(["A", tj, a_qk(tj, mo), 1280, dl_qk[(tj, mo)]])
            for tt in range(4 * tj, 4 * tj + 4):
                fq.append(["A", tj, a_v(tt), 640, dl_v[(tj, tt)]])
        fq.sort(key=lambda e: e[4])

        # debt-based pacing: optional filler is emitted (earliest deadline
        # first) while the PE work emitted so far trails the Act work
        # emitted; independently, every block is FORCED a few units before
        # its first-use unit so its DVE drain clears the queue in time.
        clock = {"pe": 0.0, "act": 0.0}

        def force_filler(until_unit):
            j = 0
            while j < len(fq):
                if fq[j][4] <= until_unit:
                    e = fq.pop(j)
                    e[2]()
                    clock["pe"] += e[3]
                else:
                    j += 1

        def drain_filler(cur_qj, budget=False, force=0):
            i = 0
            n = 0
            while i < len(fq):
                kind, idx, fn, cost = fq[i][:4]
                if n >= force and (
                        not budget or clock["pe"] >= clock["act"] - 300):
                    break
                if kind == "A" and idx > cur_qj + 1:
                    i += 1
                    continue
                fn()
                clock["pe"] += cost
                fq.pop(i)
                n += 1

        ysb_tiles = {}
        state = {}

        def sc(u):
            """Scores for unit u: per-ki trimmed matmul + PE causal mask."""
            qj, hp, lh, p = u
            s = psS.tile([128, 2, 512], F32, tag="s")
            es = pex.tile([128, 2, 512], FP8, tag="es")
            for ki in range(2):
                kt = 2 * p + ki
                r = kt - 4 * qj
                off = 128 * r if r >= 0 else 0
                nc.tensor.matmul(
                    s[:, ki, off:512],
                    qkT[2 * hp + 1][64 * lh:64 * lh + 64, kt * 128:(kt + 1) * 128],
                    qkT[2 * hp][64 * lh:64 * lh + 64,
                                qj * 512 + off:(qj + 1) * 512],
                    start=True, stop=(r < 0))
                if r >= 0:
                    # causal mask: accumulate -80 lower-triangle into the
                    # diagonal tile (exp then yields ~0; no DVE/Pool mask mul)
                    nc.tensor.matmul(
                        s[:, ki, off:off + 128], ident[:], tri[:],
                        start=False, stop=True, skip_group_check=True)
                clock["pe"] += (512 - off) * 0.4167 + (53.3 if r >= 0 else 0)
            return s, es

        pend = {}
        for i, u in enumerate(units):
            qj, hp, lh, p = u
            npair = 2 * qj + 2
            h_loc = 2 * hp + lh
            if i == 0:
                pend[0] = sc(u)
            if i + 1 < len(units):
                force_filler(i + 4)
                pend[i + 1] = sc(units[i + 1])
            s, es = pend.pop(i)
            off = 256 if p == 2 * qj + 1 else 0
            nc.scalar.activation(
                es[:, :, off:512], s[:, :, off:512], EXP, bias=negeb[:])
            clock["act"] += (1024 - 2 * off) * 0.833 + 219
            drain_filler(qj, budget=True)
            # AV (transposed, DoubleRow): psy[qt] += es_kt(qt-slice) @ (vhi|vlo)
            key = (qj, hp, lh)
            if key not in state:
                state[key] = [psY.tile([128, 512], F32, tag="psy",
                                       name=f"psy{qj}_{hp}_{lh}"), True]
            psy, first_mm = state[key]
            for qt in range(4):
                for ki in range(2):
                    kt = 2 * p + ki
                    if kt > 4 * qj + qt:
                        continue
                    es_pair = es[:, ki, qt * 128:(qt + 1) * 128] \
                        .unsqueeze(1).broadcast_to([128, 2, 128])
                    nc.tensor.matmul(
                        psy[:, qt * 128:qt * 128 + 65],
                        es_pair,
                        V[kt][:, :, h_loc, :],
                        start=first_mm,
                        stop=(kt == 4 * qj + qt),
                        perf_mode=DR,
                        skip_group_check=True)
                    first_mm = False
                    clock["pe"] += 13.5
            state[key][1] = first_mm
            if (qj, hp, lh) == (3, 1, 1) and p >= 6:
                # epilogue fast-path: normalize + transpose each qt as soon
                # as its AV accumulation stops so only qt2/qt3 trail the
                # final exp
                qts = (0, 1) if p == 6 else (2, 3)
                y_sb = ysb_tiles[qj]
                rc = pn.tile([128, 4], F32, tag="rc", name=f"rcE{p}")
                for qt in qts:
                    sb_q = pn.tile([128, 128], F32, tag="sbq", name=f"sbq{qt}")
                    nc.vector.tensor_copy(
                        sb_q[:], psy[:, qt * 128:(qt + 1) * 128])
                    nc.vector.reciprocal(rc[:, qt:qt + 1], sb_q[:, 64:65])
                    eng = nc.vector if qt == 3 else nc.gpsimd
                    eng.tensor_scalar_mul(
                        y_sb[qt][:, h_loc * 64:h_loc * 64 + 64],
                        sb_q[:, 0:64], rc[:, qt:qt + 1])
                    nc.sync.dma_start_transpose(
                        yT[1][:, qj * 512 + qt * 128:qj * 512 + (qt + 1) * 128],
                        y_sb[qt][:, 128:256])
                continue
            if p != npair - 1:
                continue
            # last pair of this head: normalize y = psy * (1/l).
            # GPSIMD can't read PSUM: drain psy to SBUF once (DVE), then
            # reciprocal + per-head muls run off SBUF (Pool-legal).
            if qj not in ysb_tiles:
                ysb_tiles[qj] = [
                    pysb.tile([128, 256], BF, tag="ysb", name=f"ysb{qj}_{q}")
                    for q in range(4)]
            y_sb = ysb_tiles[qj]
            sb_y = pn.tile([128, 512], F32, tag="sby")
            nc.vector.tensor_copy(sb_y[:], psy[:])
            rc = pn.tile([128, 4], F32, tag="rc")
            for qt in range(4):
                nc.vector.reciprocal(
                    rc[:, qt:qt + 1], sb_y[:, qt * 128 + 64:qt * 128 + 65])
            for qt in range(4):
                nc.gpsimd.tensor_scalar_mul(
                    y_sb[qt][:, h_loc * 64:h_loc * 64 + 64],
                    sb_y[:, qt * 128:qt * 128 + 64],
                    rc[:, qt:qt + 1])
            if lh == 1:
                # both heads of this pair done: transpose to yT
                for qt in range(4):
                    nc.sync.dma_start_transpose(
                        yT[hp][:, qj * 512 + qt * 128:qj * 512 + (qt + 1) * 128],
                        y_sb[qt][:, hp * 128:(hp + 1) * 128])
                if hp == 1:
                    for mo in range(8):
                        fq.append(["P", qj, proj(qj, mo), 427, 10**9])
        # drain leftover filler inside the attention scope, then run proj(3)
        # through a wide PSUM ring (psS/psY banks released) so its 8 blocks
        # stream without ring stalls
        while fq:
            fq.pop(0)[2]()
        inner.close()
        # tail proj: copies split DVE/Act, outputs staged into one tile so a
        # single strided DMA replaces 8 serialized HWDGE generations
        with tc.tile_pool(name="tail", bufs=6, space="PSUM") as ptail:
            ot_mega = po.tile([128, 8, 512], BF, tag="otm", name="ot_mega")
            # qt01 columns of yT(3) finish one pair earlier than qt23 (the
            # epilogue transposes them at p==6), so for 6 of 8 mo blocks the
            # first-half matmuls pre-run during the final exp window
            pps_t = {}
            for mo in range(6):
                pps = ptail.tile([128, 512], F32, tag="tp", name=f"tp{mo}")
                pps_t[mo] = pps
                for kt2 in range(2):
                    nc.tensor.matmul(
                        pps[:, 0:256],
                        wp[:, kt2 * C + mo * 128:kt2 * C + (mo + 1) * 128],
                        yT[kt2][:, 3 * 512:3 * 512 + 256],
                        start=(kt2 == 0), stop=(kt2 == 1),
                        skip_group_check=True)
            for mo in range(8):
                if mo < 6:
                    pps = pps_t[mo]
                    for kt2 in range(2):
                        nc.tensor.matmul(
                            pps[:, 256:512],
                            wp[:, kt2 * C + mo * 128:kt2 * C + (mo + 1) * 128],
                            yT[kt2][:, 3 * 512 + 256:4 * 512],
                            start=False, stop=(kt2 == 1),
                            skip_group_check=True)
                else:
                    pps = ptail.tile([128, 512], F32, tag="tp", name=f"tp{mo}")
                    for kt2 in range(2):
                        nc.tensor.matmul(
                            pps[:],
                            wp[:, kt2 * C + mo * 128:kt2 * C + (mo + 1) * 128],
                            yT[kt2][:, 3 * 512:4 * 512],
                            start=(kt2 == 0), stop=(kt2 == 1))
                if mo % 2 == 1:
                    nc.scalar.activation(
                        ot_mega[:, mo, :], pps[:],
                        mybir.ActivationFunctionType.Copy)
                else:
                    nc.vector.tensor_copy(ot_mega[:, mo, :], pps[:])
                # split the final DMA: the bulk streams out while the last
                # two blocks' copies finish, shortening the terminal chain
                if mo == 5:
                    nc.sync.dma_start(
                        d_out[:, 0:6, 3 * 512:4 * 512], ot_mega[:, 0:6, :])
                elif mo == 6:
                    nc.sync.dma_start(
                        d_out[:, 6, 3 * 512:4 * 512], ot_mega[:, 6, :])
            nc.sync.dma_start(
                d_out[:, 7, 3 * 512:4 * 512], ot_mega[:, 7, :])


def _build(reps=1):
    nc = bacc.Bacc("TRN2", target_bir_lowering=False, debug=False)
    d = (
        nc.dram_tensor("xhi", [128, TJ, CTP, 2, 512], FP8, kind="ExternalInput").ap(),
        nc.dram_tensor("xlo", [128, TJ, CTP, 2, 512], FP8, kind="ExternalInput").ap(),
        nc.dram_tensor("wqh", [128, 4, CTP, 2, 128], FP8, kind="ExternalInput").ap(),
        nc.dram_tensor("wql", [128, 4, CTP, 2, 128], FP8, kind="ExternalInput").ap(),
        nc.dram_tensor("wvh", [128, CTP, 2, VW], FP8, kind="ExternalInput").ap(),
        nc.dram_tensor("wvl", [128, CTP, 2, VW], FP8, kind="ExternalInput").ap(),
        nc.dram_tensor("wp", [128, 2 * C], BF, kind="ExternalInput").ap(),
        nc.dram_tensor("bqk", [128, 4], F32, kind="ExternalInput").ap(),
        nc.dram_tensor("tri", [128, 128], FP8, kind="ExternalInput").ap(),
        nc.dram_tensor("ident", [128, 128], FP8, kind="ExternalInput").ap(),
        nc.dram_tensor("outT", [128, 8, T], BF, kind="ExternalOutput").ap(),
    )
    with tile.TileContext(nc) as tc:
        for rep in range(reps):
            if rep:
                tc.strict_bb_all_engine_barrier()
            _emit(tc, nc, d)
    nc.compile()
    return nc


def _sb(a):
    """[128k, n] -> SBUF layout [128, k, n] (k-tile-major along free dim)."""
    k = a.shape[0] // 128
    return np.ascontiguousarray(
        a.reshape(k, 128, a.shape[1]).transpose(1, 0, 2))


def _f8(a):
    return np.ascontiguousarray(a).astype(NFP8)


def _split8(a):
    """f32 -> (hi, lo) fp8 residual split."""
    hi = np.asarray(a, np.float32).astype(NFP8)
    lo = (np.asarray(a, np.float32) - hi.astype(np.float32)).astype(NFP8)
    return hi, lo


def _wqk_layout(w):
    """[C, 512] -> [128, mo, ctp, 2, 128] (ct pairs interleaved for DR)."""
    s = _sb(w)                                   # [128, ct(8), 512]
    s = s.reshape(128, CTP, 2, 4, 128).transpose(0, 3, 1, 2, 4)
    return np.ascontiguousarray(s)


def _x_layout(xT):
    """[128, ct, T] -> [128, tj, ctp, 2, 512]."""
    s = xT.reshape(128, CTP, 2, TJ, 512).transpose(0, 3, 1, 2, 4)
    return np.ascontiguousarray(s)


def _wv_layout(w):
    """[C, 256] -> [128, ctp, 2, 256]."""
    s = _sb(w)                                   # [128, ct, 256]
    s = s.reshape(128, CTP, 2, VW)
    return np.ascontiguousarray(s)


def _prep_in_maps(inputs):
    x = np.asarray(inputs["x"], np.float32)
    W_attn = np.asarray(inputs["W_attn"], np.float32)
    b_attn = np.asarray(inputs["b_attn"], np.float32)
    W_proj = np.asarray(inputs["W_proj"], np.float32)

    scale = 1.0 / np.sqrt(HD)
    # -80 lower-triangle (mask k>q within the diagonal tile: j < p)
    tri = np.where(np.arange(128)[None, :] < np.arange(128)[:, None],
                   -80.0, 0.0).astype(np.float32)
    ident = np.eye(128, dtype=np.float32)

    in_maps = []
    for c in range(NCORES):
        b, g = divmod(c, 4)
        heads = [4 * g + i for i in range(HPC)]
        xT = _sb(np.ascontiguousarray(x[b].T))              # [128, ct, 2048]
        xh, xl = _split8(xT)
        xh = _x_layout(xh.astype(np.float32)).astype(NFP8)
        xl = _x_layout(xl.astype(np.float32)).astype(NFP8)

        wq = [W_attn[:, h * HD:(h + 1) * HD] * (scale * SQ) for h in heads]
        wk = [W_attn[:, C + h * HD:C + (h + 1) * HD] * SK for h in heads]
        wqk = np.concatenate(
            [wq[0], wq[1], wk[0], wk[1], wq[2], wq[3], wk[2], wk[3]], axis=1)
        wqk_hi, wqk_lo = _split8(wqk)
        wqh = _wqk_layout(wqk_hi.astype(np.float32)).astype(NFP8)
        wql = _wqk_layout(wqk_lo.astype(np.float32)).astype(NFP8)

        wv = np.concatenate(
            [W_attn[:, 2 * C + h * HD:2 * C + (h + 1) * HD] for h in heads],
            axis=1) * SV                                    # [C, 256]
        wv_hi, wv_lo = _split8(wv)
        wvh = _wv_layout(wv_hi.astype(np.float32)).astype(NFP8)
        wvl = _wv_layout(wv_lo.astype(np.float32)).astype(NFP8)

        wp = np.zeros((128, 2 * C), np.float32)
        for kt2 in range(2):
            rows = np.concatenate(
                [W_proj[heads[2 * kt2 + j] * HD:(heads[2 * kt2 + j] + 1) * HD, :]
                 for j in range(2)], axis=0)                # [128, 1024]
            wp[:, kt2 * C:(kt2 + 1) * C] = rows

        bqk = np.zeros((128, 4), np.float32)
        for i2 in range(2):   # head pair
            for j in range(2):
                h = heads[2 * i2 + j]
                bqk[64 * j:64 * j + 64, 2 * i2] = b_attn[h * HD:(h + 1) * HD] * scale
                bqk[64 * j:64 * j + 64, 2 * i2 + 1] = b_attn[C + h * HD:C + (h + 1) * HD]

        in_maps.append({"xhi": xh, "xlo": xl, "wqh": wqh, "wql": wql,
                        "wvh": wvh, "wvl": wvl,
                        "wp": np.ascontiguousarray(wp).astype(ml_dtypes.bfloat16),
                        "bqk": bqk, "tri": _f8(tri), "ident": _f8(ident)})
    return in_maps


def kernel(x, W_attn, b_attn, W_proj, b_proj):
    in_maps = _prep_in_maps(dict(x=x, W_attn=W_attn, b_attn=b_attn,
                                 W_proj=W_proj, b_proj=b_proj))
    if "nc" not in _CACHE:
        _CACHE["nc"] = _build()
    nc = _CACHE["nc"]
    res = run_bass_kernel_spmd(nc, in_maps, core_ids=list(range(NCORES)))

    out = np.zeros((B, T, C), np.float32)
    for c in range(NCORES):
        b = c // 4
        oT = np.asarray(res.results[c]["outT"], np.float32)         # [128, 8*2048]
        oT = oT.reshape(128, 8, T).transpose(1, 0, 2).reshape(C, T)  # [C, T]
        out[b] += oT.T
    # bv was dropped from the device V path: y_true = y_dev + bv, so
    # out_true = out_dev + bv @ W_proj (+ b_proj), both added here.
    bv_full = np.asarray(b_attn, np.float32)[2 * C:3 * C]
    out += (bv_full @ np.asarray(W_proj, np.float32))[None, None, :]
    out += np.asarray(b_proj, np.float32)[None, None, :]
    return out
